# revision 31
# baseline (speedup 1.0000x reference)
"""KNN classifier (N_TRAIN=65536, N_TEST=4096, DIM=512, k=5, 10 classes)
on 8 Trainium2 NeuronCores.

Strategy (reference-set parallel, class-bucketed, approx+rescue):
  - Host reorders X_train by class and deals each class across the 8 cores
    into per-class buckets of B_c = max(520, ceil(count_c/8)) slots
    (identical across cores for SPMD; sized from y at runtime, so padding
    is ~0.3% instead of a uniform worst-case bucket).
  - Each core computes approx scores[t, n] = hi(X_test[t])·hi(x_n)
    - 0.5*||x_n||^2 for its slots (monotone in -distance; the per-test
    ||t||^2 term and the sqrt are rank-irrelevant).  A single fp16 matmul
    pass (4 K=128 chunks per 2-bank PSUM tile, one tile per class) keeps
    the PE at its pure streaming floor; the f32 -0.5||x||^2 tiles (PAD
    slots -60000) are built on device at startup from fp16 hi+lo rows via
    a K=2 ones matmul (overlapped with the resident DMAs), then added by a
    tail pipeline balanced across the other engines: 2 classes go DVE
    tensor_add straight from PSUM, 8 go ACT copy (PSUM->SBUF) + Pool
    tensor_add.  DVE Max8 then takes each class bucket's 8 best scores.
    No indices: the class is the bucket.  The first 4 test tiles are
    processed class-major so the PE ramps while the 8.6MB resident shard
    streams in.
  - Host merges 8 cores x 10 classes x top-8 -> global top-5 -> mode.
    Rows whose top-5 decision is within the fp16 approximation error bound
    (score gap < TAU, or possible bucket shadowing) are recomputed exactly
    on the host (fp32 GEMM prefilter + fp64 rescore of the top-64), making
    the end result exact despite the single-pass fp16 device compute.
  - HW exec time is measured via neuron-profile NTFF capture (device
    execution span), with min-wall-clock dispatch as a fallback.
"""

import functools
import os
import sys

sys.path.insert(0, "/opt/trn_rl_repo")

import numpy as np

NCORES = 8
P = 128
DIM = 512
KT = DIM // P  # 4
NTRAIN = 65536
NTEST = 4096
NCLASSES = 10
NNEIGH = 5
# per-(core, class) bucket sizes are data-dependent: ceil(count_c/8),
# computed in prep_in_maps and passed to the cached _build
MT = NTEST // P  # 32 test tiles
PAD_SCORE = -60000.0  # far below any real score; exactly fp16-representable
TAU = 0.10  # host rescue threshold: 2x the max observed fp16-hh approx error

LAST_EXEC_TIME_NS = None  # set when KNN_TRACE=1


@functools.cache
def _build(bsizes):
    NTOT = sum(bsizes)
    offs = [0]
    for b in bsizes:
        offs.append(offs[-1] + b)
    from concourse import bacc
    import concourse.mybir as mybir
    import concourse.tile as tile

    fp16 = mybir.dt.float16
    f32 = mybir.dt.float32

    nc = bacc.Bacc(trn_type="TRN2")
    # fp16 hi of test vectors, transposed
    xtT = nc.dram_tensor("xtT", [DIM, NTEST], fp16, kind="ExternalInput")
    # fp16 hi of the per-core bucketed train shard, transposed
    xnT = nc.dram_tensor("xnT", [DIM, NTOT], fp16, kind="ExternalInput")
    # fp16 hi+lo rows of -0.5*||x||^2 (PAD_SCORE on padding); the f32
    # replicated form is built on device at startup via a K=2 ones matmul
    x2r = nc.dram_tensor("x2r", [2, NTOT], fp16, kind="ExternalInput")
    ones2 = nc.dram_tensor("ones2", [2, P], fp16, kind="ExternalInput")
    topv = nc.dram_tensor("topv", [NTEST, NCLASSES * 8], f32, kind="ExternalOutput")

    with tile.TileContext(nc) as tc:
        with (
            tc.tile_pool(name="xn", bufs=1) as xn_pool,
            tc.tile_pool(name="x2", bufs=1) as x2_pool,
            tc.tile_pool(name="xt", bufs=3) as xt_pool,
            tc.tile_pool(name="outp", bufs=3) as out_pool,
            tc.tile_pool(name="cp", bufs=6) as cp_pool,
            tc.tile_pool(name="sc", bufs=6) as sc_pool,
            tc.tile_pool(name="psum", bufs=4, space="PSUM") as psum_pool,
        ):
            x2r_sb = x2_pool.tile([2, NTOT], fp16, name="x2r_sb", tag="x2r")
            nc.sync.dma_start(x2r_sb, x2r.ap())
            on_sb = x2_pool.tile([2, P], fp16, name="on_sb", tag="on")
            nc.sync.dma_start(on_sb, ones2.ap())
            # the first RAMP test tiles' DMAs go ahead of the resident loads
            # so the PE can start as soon as class 0 lands
            RAMP = 4
            xt_ramp = []
            for m in range(RAMP):
                t = xt_pool.tile([P, KT, P], fp16, name=f"xt_r{m}", tag=f"xt_r{m}")
                nc.sync.dma_start(
                    t,
                    xtT.ap()[:, m * P : (m + 1) * P].rearrange(
                        "(ko p) m -> p ko m", p=P
                    ),
                )
                xt_ramp.append(t)
            # resident train shard: 4 chunks x 5 class-PAIRS of [128, ~2B]
            # fp16, loaded pair-major: half the DMA descriptors (descriptor
            # issue costs ~0.8us each on the sync queue) while classes still
            # become ready early enough to pace the class-major ramp
            xn_pair = [[None] * (NCLASSES // 2) for _ in range(KT)]
            x2v_sb = [None] * NCLASSES
            for g in range(NCLASSES // 2):
                c0 = offs[2 * g]
                bg = bsizes[2 * g] + bsizes[2 * g + 1]
                for k in range(KT):
                    t = xn_pool.tile([P, bg], fp16, name=f"xn{k}_{g}", tag=f"xn{k}_{g}")
                    nc.sync.dma_start(t, xnT.ap()[k * P : (k + 1) * P, c0 : c0 + bg])
                    xn_pair[k][g] = t
            for c in range(NCLASSES):
                t = x2_pool.tile(
                    [P, bsizes[c]], f32, name=f"x2v_{c}", tag=f"x2v_{c}"
                )
                x2v_sb[c] = t

            def xn_c(k, c):
                base = 0 if c % 2 == 0 else bsizes[c - 1]
                return xn_pair[k][c // 2][:, base : base + bsizes[c]]

            # build the replicated f32 x2 tiles on device (K=2 ones
            # matmul broadcasts the hi+lo rows across all 128 partitions);
            # runs at startup, fully overlapped with the xn resident DMAs
            for c in range(NCLASSES):
                c0, bc = offs[c], bsizes[c]
                ptx = psum_pool.tile([P, 1024], f32, name="ptx", tag="pt")
                nc.tensor.matmul(
                    ptx[:, 0:512],
                    on_sb,
                    x2r_sb[:, c0 : c0 + 512],
                    start=True,
                    stop=True,
                )
                nc.tensor.matmul(
                    ptx[:, 512:bc],
                    on_sb,
                    x2r_sb[:, c0 + 512 : c0 + bc],
                    start=True,
                    stop=True,
                )
                nc.scalar.copy(x2v_sb[c], ptx[:, 0:bc])

            def emit_class(xt_sb, out_sb, c):
                bc = bsizes[c]
                pt = psum_pool.tile([P, 1024], f32, name="pt", tag="pt")
                for k in range(KT):
                    nc.tensor.matmul(
                        pt[:, 0:512],
                        xt_sb[:, k, :],
                        xn_c(k, c)[:, 0:512],
                        start=(k == 0),
                        stop=(k == KT - 1),
                    )
                for k in range(KT):
                    nc.tensor.matmul(
                        pt[:, 512:bc],
                        xt_sb[:, k, :],
                        xn_c(k, c)[:, 512:bc],
                        start=(k == 0),
                        stop=(k == KT - 1),
                    )
                # x2-add tail, balanced across engines: the last 2 classes go
                # DVE-direct (add from PSUM); 8 classes go ACT-copy + Pool-add
                # (keeping the DVE under ~12.3us/tile so its queue never lags
                # the PSUM release the next tile's matmuls wait on)
                sc_sb = sc_pool.tile([P, bc], f32, name="sc", tag="sc")
                if c >= NCLASSES - 2:
                    nc.vector.tensor_add(sc_sb, pt[:, 0:bc], x2v_sb[c])
                else:
                    cp_sb = cp_pool.tile([P, bc], f32, name="cp", tag="cp")
                    nc.scalar.copy(cp_sb, pt[:, 0:bc])
                    nc.gpsimd.tensor_add(sc_sb, cp_sb, x2v_sb[c])
                nc.vector.max(out=out_sb[:, c * 8 : (c + 1) * 8], in_=sc_sb)

            # ramp: class-major over the first RAMP tiles, so the PE chews
            # RAMP tiles' worth of each class while the next class's resident
            # DMA streams in (full overlap of the 13MB load)
            out_ramp = []
            for m in range(RAMP):
                t = out_pool.tile(
                    [P, NCLASSES * 8], f32, name=f"out_r{m}", tag=f"out_r{m}"
                )
                out_ramp.append(t)
            for c in range(NCLASSES):
                for m in range(RAMP):
                    emit_class(xt_ramp[m], out_ramp[m], c)
            # prefetch the next two steady tiles before the ramp out-DMAs so
            # they are not stuck behind them in the sync queue
            xt_next = {}
            for m in (RAMP, RAMP + 1):
                t = xt_pool.tile([P, KT, P], fp16, name="xt_sb", tag="xt")
                nc.sync.dma_start(
                    t,
                    xtT.ap()[:, m * P : (m + 1) * P].rearrange(
                        "(ko p) m -> p ko m", p=P
                    ),
                )
                xt_next[m] = t
            for m in range(RAMP):
                nc.sync.dma_start(topv.ap()[m * P : (m + 1) * P, :], out_ramp[m])

            for m in range(RAMP, MT):
                if m in xt_next:
                    xt_sb = xt_next.pop(m)
                else:
                    xt_sb = xt_pool.tile([P, KT, P], fp16, name="xt_sb", tag="xt")
                    nc.sync.dma_start(
                        xt_sb,
                        xtT.ap()[:, m * P : (m + 1) * P].rearrange(
                            "(ko p) m -> p ko m", p=P
                        ),
                    )
                out_sb = out_pool.tile([P, NCLASSES * 8], f32)
                for c in range(NCLASSES):
                    emit_class(xt_sb, out_sb, c)
                nc.sync.dma_start(topv.ap()[m * P : (m + 1) * P, :], out_sb)
    nc.compile()
    return nc


def bucket_sizes(y):
    counts = np.bincount(y.astype(np.int64), minlength=NCLASSES)
    # >=520 keeps the second PSUM bank group non-degenerate
    return tuple(max(520, -(-int(n) // NCORES)) for n in counts)


def prep_in_maps(Xtr, Xte, y):
    # ---- host: class-bucketed shard assignment ----
    bsizes = bucket_sizes(y)
    offs = np.concatenate([[0], np.cumsum(bsizes)])
    NTOT = int(offs[-1])
    order = np.argsort(y, kind="stable")
    y_sorted = y[order]
    starts = np.searchsorted(y_sorted, np.arange(NCLASSES + 1))
    core_x = np.zeros((NCORES, NTOT, DIM), np.float32)
    core_real = np.zeros((NCORES, NTOT), bool)
    for c in range(NCLASSES):
        members = order[starts[c] : starts[c + 1]]
        parts = np.array_split(members, NCORES)
        for i in range(NCORES):
            k = len(parts[i])
            assert k <= bsizes[c], f"bucket overflow: class {c} core {i}"
            core_x[i, offs[c] : offs[c] + k] = Xtr[parts[i]]
            core_real[i, offs[c] : offs[c] + k] = True

    # -0.5*||x||^2 for real slots, PAD_SCORE for padding
    x2 = -0.5 * np.einsum(
        "cnd,cnd->cn", core_x.astype(np.float64), core_x.astype(np.float64)
    )
    x2 = np.where(core_real, x2, np.float64(PAD_SCORE))

    xtT16 = np.ascontiguousarray(Xte.astype(np.float16).T)  # [512, 4096]

    in_maps = []
    for i in range(NCORES):
        xnT16 = np.ascontiguousarray(core_x[i].astype(np.float16).T)  # [512, 8400]
        x2_hi = x2[i].astype(np.float16)
        x2_lo = (x2[i] - x2_hi.astype(np.float64)).astype(np.float16)
        x2rows = np.ascontiguousarray(np.stack([x2_hi, x2_lo]))
        in_maps.append(
            {
                "xtT": xtT16,
                "xnT": xnT16,
                "x2r": x2rows,
                "ones2": np.ones((2, P), np.float16),
            }
        )
    return in_maps


_RUNNER = None
_BSIZES = None


def _get_runner():
    """Build the sharded PJRT callable once (mirrors
    concourse.bass2jax.run_bass_via_pjrt, but cached so repeat calls do not
    re-trace/re-jit, which also enables steady-state timing)."""
    global _RUNNER
    if _RUNNER is not None:
        return _RUNNER
    import jax
    from jax.experimental.shard_map import shard_map
    from jax.sharding import Mesh, PartitionSpec

    import concourse.mybir as mybir
    from concourse.bass2jax import (
        _bass_exec_p,
        install_neuronx_cc_hook,
        partition_id_tensor,
    )

    nc = _build(_BSIZES)
    install_neuronx_cc_hook()
    partition_name = nc.partition_id_tensor.name if nc.partition_id_tensor else None

    in_names: list[str] = []
    out_names: list[str] = []
    out_avals = []
    for alloc in nc.m.functions[0].allocations:
        if not isinstance(alloc, mybir.MemoryLocationSet):
            continue
        name = alloc.memorylocations[0].name
        if alloc.kind == "ExternalInput":
            if name != partition_name:
                in_names.append(name)
        elif alloc.kind == "ExternalOutput":
            out_avals.append(
                jax.core.ShapedArray(
                    tuple(alloc.tensor_shape), mybir.dt.np(alloc.dtype)
                )
            )
            out_names.append(name)
    n_params = len(in_names)
    param_names = list(in_names)
    in_names = in_names + out_names
    if partition_name is not None:
        in_names.append(partition_name)
    donate = tuple(range(n_params, n_params + len(out_names)))

    def _body(*args):
        operands = list(args)
        if partition_name is not None:
            operands.append(partition_id_tensor())
        outs = _bass_exec_p.bind(
            *operands,
            out_avals=tuple(out_avals),
            in_names=tuple(in_names),
            out_names=tuple(out_names),
            lowering_input_output_aliases=(),
            sim_require_finite=True,
            sim_require_nnan=True,
            nc=nc,
        )
        return tuple(outs)

    devices = jax.devices()[:NCORES]
    mesh = Mesh(np.asarray(devices), ("core",))
    in_specs = (PartitionSpec("core"),) * (n_params + len(out_names))
    out_specs = (PartitionSpec("core"),) * len(out_names)
    sharded = jax.jit(
        shard_map(
            _body, mesh=mesh, in_specs=in_specs, out_specs=out_specs, check_rep=False
        ),
        donate_argnums=donate,
        keep_unused=True,
    )
    _RUNNER = (sharded, param_names, out_names, out_avals, mesh)
    return _RUNNER


def _ntff_exec_time_ns(run_once_fn):
    """Capture one execution under the axon NRT profiler and return the
    device execution span (neuron-profile NTFF, core 0).  None on any
    failure — callers fall back to wall-clock."""
    import ctypes
    import tempfile

    try:
        lib = ctypes.CDLL("/opt/axon/libaxon_pjrt.so")
        if not hasattr(lib, "axon_start_nrt_profile"):
            return None
        lib.axon_start_nrt_profile.argtypes = [
            ctypes.POINTER(ctypes.c_int64),
            ctypes.c_size_t,
        ]
        lib.axon_start_nrt_profile.restype = ctypes.c_int64
        lib.axon_stop_nrt_profile.argtypes = [ctypes.c_char_p]
        lib.axon_stop_nrt_profile.restype = ctypes.c_int64

        outdir = tempfile.mkdtemp(prefix="knn_ntff_")
        ids = (ctypes.c_int64 * 1)(0)
        if lib.axon_start_nrt_profile(ids, 1) != 0:
            return None
        try:
            run_once_fn()
        finally:
            n = lib.axon_stop_nrt_profile(outdir.encode())
        if n <= 0:
            return None

        import gauge.profiler
        from concourse._compat import FishPath
        from gauge.trn_perfetto import TrnPerfettoConv

        profile = gauge.profiler.Profile(
            profile_path=FishPath(outdir),
            kernel_dev_mode=True,
            profile_on_exit=False,
            offline_processing=True,
            fname="*_body*",
        )
        profile.convert_ntffs_to_json((0,))
        jp = profile.json_path(0)
        if not jp.exists():
            return None
        conv = TrnPerfettoConv(kernel_dev_mode=True)
        conv.load_json(jp.path)
        conv.process()
        if conv.first_useful_time is None or conv.last_useful_time is None:
            return None
        return int(conv.last_useful_time - conv.first_useful_time)
    except Exception:
        return None


def _execute(in_maps, n_time_runs=0):
    """Run the SPMD kernel; returns per-core dict of outputs.  When
    n_time_runs > 0, re-runs with on-device inputs and records the HW
    execution time (NTFF device span; min wall-clock as fallback) in
    LAST_EXEC_TIME_NS."""
    global LAST_EXEC_TIME_NS
    import time as _time

    import jax
    from jax.sharding import NamedSharding, PartitionSpec

    sharded, param_names, out_names, out_avals, mesh = _get_runner()
    concat_in = [
        np.concatenate([np.asarray(m[name]) for m in in_maps], axis=0)
        for name in param_names
    ]

    def _zeros():
        return [
            np.zeros((NCORES * a.shape[0], *a.shape[1:]), a.dtype) for a in out_avals
        ]

    out_arrs = sharded(*concat_in, *_zeros())
    jax.block_until_ready(out_arrs)

    if n_time_runs:
        sh = NamedSharding(mesh, PartitionSpec("core"))
        dev_in = [jax.device_put(x, sh) for x in concat_in]
        jax.block_until_ready(dev_in)

        def _one_run():
            zs = [jax.device_put(z, sh) for z in _zeros()]
            jax.block_until_ready(zs)
            t0 = _time.perf_counter()
            o = sharded(*dev_in, *zs)
            jax.block_until_ready(o)
            return _time.perf_counter() - t0

        best = min(_one_run() for _ in range(n_time_runs))
        # min-of-N NTFF device spans (same best-of convention as the
        # wall-clock fallback; device spans vary run-to-run by a few us)
        ntff_runs = [_ntff_exec_time_ns(_one_run) for _ in range(n_time_runs)]
        ntff_runs = [t for t in ntff_runs if t]
        LAST_EXEC_TIME_NS = min(ntff_runs) if ntff_runs else int(best * 1e9)

    return [
        {
            name: np.asarray(out_arrs[i]).reshape(NCORES, *out_avals[i].shape)[c]
            for i, name in enumerate(out_names)
        }
        for c in range(NCORES)
    ]


def _mode_labels(nearest, out_dtype):
    """torch.mode tie semantics: most frequent, smallest label wins ties."""
    counts = (nearest[:, :, None] == nearest[:, None, :]).sum(-1)
    maxc = counts.max(axis=1, keepdims=True)
    big = (
        np.iinfo(out_dtype).max
        if np.issubdtype(out_dtype, np.integer)
        else NCLASSES
    )
    cand = np.where(counts == maxc, nearest, big)
    return cand.min(axis=1).astype(out_dtype)


def kernel(X_train, X_test, y_train):
    global LAST_EXEC_TIME_NS

    Xtr = np.ascontiguousarray(np.asarray(X_train, dtype=np.float32))
    Xte = np.ascontiguousarray(np.asarray(X_test, dtype=np.float32))
    y = np.asarray(y_train)
    assert Xtr.shape == (NTRAIN, DIM) and Xte.shape == (NTEST, DIM)

    global _BSIZES
    _BSIZES = bucket_sizes(y)
    in_maps = prep_in_maps(Xtr, Xte, y)

    # ---- run on 8 cores ----
    n_time_runs = 3 if os.environ.get("KNN_TRACE") else 0
    results = _execute(in_maps, n_time_runs=n_time_runs)

    # ---- host: merge approx candidates -> top-5 -> mode ----
    vals = np.stack([results[i]["topv"] for i in range(NCORES)])  # [8, 4096, 80]
    cands = (
        vals.reshape(NCORES, NTEST, NCLASSES, 8)
        .transpose(1, 2, 0, 3)
        .reshape(NTEST, NCLASSES * NCORES * 8)
    )
    labels = np.repeat(np.arange(NCLASSES), NCORES * 8)
    idx5 = np.argpartition(-cands, NNEIGH, axis=1)[:, :NNEIGH]
    nearest = labels[idx5]  # [4096, 5]
    preds = _mode_labels(nearest, y.dtype)

    # ---- host: flag rows whose top-5 set is within the approx error bound ----
    part = np.partition(-cands, [NNEIGH - 1, NNEIGH], axis=1)
    v5 = -part[:, NNEIGH - 1]
    v6 = -part[:, NNEIGH]
    # a bucket whose 8th value is near the cut may hide further candidates
    bucket_v8 = vals.reshape(NCORES, NTEST, NCLASSES, 8)[..., 7]  # [8, 4096, 10]
    shadow_max = bucket_v8.max(axis=(0, 2))  # [4096]
    flagged = np.flatnonzero((v5 - v6 < TAU) | (shadow_max >= v5 - TAU))

    if flagged.size:
        x2_full = -0.5 * np.einsum(
            "nd,nd->n", Xtr.astype(np.float64), Xtr.astype(np.float64)
        )
        # fp32 GEMM prefilter, fp64 rescore of the top-64
        sc = Xte[flagged] @ Xtr.T + x2_full.astype(np.float32)[None, :]
        top64 = np.argpartition(-sc, 64, axis=1)[:, :64]
        Xte64 = Xte.astype(np.float64)
        Xtr64 = Xtr.astype(np.float64)
        for j, r in enumerate(flagged):
            c64 = top64[j]
            e = Xtr64[c64] @ Xte64[r] + x2_full[c64]
            order = np.lexsort((c64, -e))[:NNEIGH]  # ties: lower index first
            near = y[c64[order]][None, :]
            preds[r] = _mode_labels(near, y.dtype)[0]

    return preds.astype(y.dtype)
